# revision 5
# baseline (speedup 1.0000x reference)
"""ACE loss kernel for TRN2, data-parallel over 8 NeuronCores.

Math (per sample b, with targets y[b, 0:8] and logits x[b, c, t]):
  m[b,t]   = max_c x[b,c,t]
  cnt[b,j] = #{t : x[b, y[b,j], t] == m[b,t]}        == n_k[b, y[b,j]] (no ties)
  dup[b,j] = multiplicity of y[b,j] within y[b,:]    == y_k[b, y[b,j]]
Only target classes contribute to the masked loss, so the full 128-bin
argmax histogram is never materialized:
  n_sum[b] = sum_j cnt/dup   (each distinct class counted once)
  n_p[b,j] = max(cnt / max(n_sum,1), EPS)
  loss[b]  = sum_j n_p * (-log(dup/8)) / dup
  out      = mean_b loss

v3: x streams through HWDGE (sync-engine ring) in raw f32 at the
measured ~434 GB/s fabric/HBM-read ceiling (SWDGE descriptor generation
starves under DVE shared-port locks -- the v1 pathology; HWDGE is RTL
and immune).  yc/xg ride the SAME ring, sequenced between tile-0's
quarters and tile 1: v2 put them on the concurrent scalar ring, which
round-robins per packet with Q1 and collapsed both rings to ~160 GB/s
for 8 us.  Tiles 1-4 are cast f32->fp16 by the Scalar/ACT engine
(dedicated SBUF ports, otherwise idle) and max-reduced by DVE in fp16
2x mode; tiles 0,5,6,7 do the first tree level as a DVE f32 TT-max
with fp16 output (monotone cast -> same numerics), which has lower
latency than cast-then-tree and so suits the stream's head and tail.
The last tile loads as 2 quarters + 4 eighths with a running fold so
only ~1.6 us of tree work follows the final bytes.  One batched
epilogue covers all 8 tile columns, the [128,1] loss vector collapses
via a PE dot with ones, and the scalar leaves through a 4-byte
single-descriptor DMA.  Each core returns one f32; the host sums 8 and
divides by B.
"""

import numpy as np

B, C, T, L = 8192, 128, 64, 8
N_CORES = 8
B_SH = B // N_CORES          # 1024 samples per core
NT = B_SH // 128             # 8 tiles of 128 samples
EPS = 1e-5

_CACHE = {}


def _build_nc():
    import os
    import sys
    if "/opt/trn_rl_repo" not in sys.path:
        sys.path.insert(0, "/opt/trn_rl_repo")
    from concourse import bacc, mybir
    from concourse.tile import TileContext

    f32 = mybir.dt.float32
    f16 = mybir.dt.float16
    AX = mybir.AxisListType
    OP = mybir.AluOpType

    CT = C * T            # 8192 elems per sample
    Q = CT // 4           # 2048
    E = CT // 8           # 1024
    H = CT // 2           # 4096

    nc = bacc.Bacc("TRN2")
    x = nc.declare_dram_parameter("x", [B_SH, CT], f32, isOutput=False)
    xg = nc.declare_dram_parameter("xg", [128, NT * L * T], f16, isOutput=False)
    yc = nc.declare_dram_parameter("yc", [128, NT * L], mybir.dt.int32, isOutput=False)
    out = nc.declare_dram_parameter("out", [1, 1], f32, isOutput=True)

    with TileContext(nc) as tc:
        with (
            tc.tile_pool(name="xp", bufs=4) as xp,
            tc.tile_pool(name="xh", bufs=2) as xhp,
            tc.tile_pool(name="hp", bufs=2) as hp,
            tc.tile_pool(name="sp", bufs=2) as sp,
            tc.tile_pool(name="cp", bufs=1) as cp,
            tc.tile_pool(name="ps", bufs=1, space="PSUM") as pp,
        ):
            # ---- whole-run tiles ----
            xga = cp.tile([128, NT * L * T], f16)
            ycta = cp.tile([128, NT * L], mybir.dt.int32)
            mh = cp.tile([128, NT * T], f16)      # per-tile class-max rows
            cnta = cp.tile([128, NT * L], f32)
            ones = cp.tile([128, 1], f32)
            nc.gpsimd.memset(ones[:, :], 1.0)

            ycf = cp.tile([128, NT * L], f32)
            eq8 = cp.tile([128, NT * L * L], f32)
            dup = cp.tile([128, NT * L], f32)
            rd = cp.tile([128, NT * L], f32)
            lg = cp.tile([128, NT * L], f32)
            wgt = cp.tile([128, NT * L], f32)

            def ymath():
                nc.vector.tensor_copy(out=ycf[:, :], in_=ycta[:, :])
                nc.vector.tensor_tensor(
                    out=eq8[:, :].rearrange("p (k a b) -> p k a b", a=L, b=L),
                    in0=ycf[:, :].rearrange("p (k a) -> p k a", a=L)
                    .unsqueeze(3).to_broadcast([128, NT, L, L]),
                    in1=ycf[:, :].rearrange("p (k a) -> p k a", a=L)
                    .unsqueeze(2).to_broadcast([128, NT, L, L]),
                    op=OP.is_equal,
                )
                nc.vector.reduce_sum(
                    out=dup[:, :],
                    in_=eq8[:, :].rearrange("p (k a b) -> p k a b", a=L, b=L),
                    axis=AX.X,
                )
                nc.vector.reciprocal(out=rd[:, :], in_=dup[:, :])
                nc.scalar.activation(
                    out=lg[:, :], in_=dup[:, :],
                    func=mybir.ActivationFunctionType.Ln, scale=1.0 / L,
                )

            # fp16 in-place max tree over t[:, lo:lo+n] down to T wide
            def tree16(t, lo, n):
                w = n
                while w > T:
                    h = w // 2
                    nc.vector.tensor_tensor(
                        out=t[:, lo:lo + h], in0=t[:, lo:lo + h],
                        in1=t[:, lo + h:lo + w], op=OP.max,
                    )
                    w = h

            def count(k):
                eq = sp.tile([128, L * T], f16, tag="eq")
                nc.vector.tensor_tensor(
                    out=eq[:, :].rearrange("p (l t) -> p l t", l=L),
                    in0=xga[:, k * L * T:(k + 1) * L * T].rearrange(
                        "p (l t) -> p l t", l=L
                    ),
                    in1=mh[:, k * T:(k + 1) * T].unsqueeze(1)
                    .to_broadcast([128, L, T]),
                    op=OP.is_equal,
                )
                nc.vector.reduce_sum(
                    out=cnta[:, k * L:(k + 1) * L],
                    in_=eq[:, :].rearrange("p (l t) -> p l t", l=L),
                    axis=AX.X,
                )

            for k in range(NT):
                row = slice(k * 128, (k + 1) * 128)
                xt = xp.tile([128, CT], f32, tag="xt")
                if k == 0 or k == NT - 1:
                    # DVE-L1 pieces: quarters for t0 (short ramp); quarters
                    # then eighths for t7 (short drain tail).  Each piece:
                    # f32 pair-max with fp16 out, fp16 sub-tree, running fold.
                    ht = hp.tile([128, H], f16, tag="ht")
                    pieces = ([(c * Q, Q) for c in range(4)] if k == 0 else
                              [(0, Q), (Q, Q)] +
                              [(H + c * E, E) for c in range(4)])
                    hoff = 0
                    for pi, (lo, n) in enumerate(pieces):
                        nc.sync.dma_start(
                            out=xt[:, lo:lo + n], in_=x[row, lo:lo + n]
                        )
                        if k == 0 and pi == len(pieces) - 1:
                            # yc + xg ride the x ring here, serialized after
                            # the quarters: on the concurrent scalar ring
                            # they halve Q1 throughput for ~8 us
                            nc.sync.dma_start(out=ycta[:, :], in_=yc[:, :])
                            nc.sync.dma_start(out=xga[:, :], in_=xg[:, :])
                        last = pi == len(pieces) - 1
                        nc.vector.tensor_tensor(
                            out=ht[:, hoff:hoff + n // 2],
                            in0=xt[:, lo:lo + n // 2],
                            in1=xt[:, lo + n // 2:lo + n],
                            op=OP.max,
                        )
                        tree16(ht, hoff, n // 2)
                        if pi > 0:
                            # fold into the running max (last fold -> mh)
                            dst = (mh[:, k * T:(k + 1) * T] if last
                                   else ht[:, 0:T])
                            nc.vector.tensor_tensor(
                                out=dst, in0=ht[:, 0:T],
                                in1=ht[:, hoff:hoff + T], op=OP.max,
                            )
                        if k == 0 and last:
                            # yc landed right after the quarters; y-side math
                            # fills the DVE gap before xga arrives
                            ymath()
                        hoff += n // 2
                else:
                    halves_on_act = k <= 4
                    if halves_on_act:
                        # ACT casts each half as it lands; DVE runs the pure
                        # fp16 sub-tree per half (2x mode) + fold
                        xh = xhp.tile([128, CT], f16, tag="xh")
                        for c in range(2):
                            sl = slice(c * H, (c + 1) * H)
                            nc.sync.dma_start(out=xt[:, sl], in_=x[row, sl])
                            nc.scalar.copy(out=xh[:, sl], in_=xt[:, sl])
                            tree16(xh, c * H, H)
                        nc.vector.tensor_tensor(
                            out=mh[:, k * T:(k + 1) * T], in0=xh[:, 0:T],
                            in1=xh[:, H:H + T], op=OP.max,
                        )
                    else:
                        # stream tail transition: DVE-L1 halves (lower
                        # latency than cast-then-tree)
                        ht = hp.tile([128, H], f16, tag="ht")
                        for c in range(2):
                            sl = slice(c * H, (c + 1) * H)
                            nc.sync.dma_start(out=xt[:, sl], in_=x[row, sl])
                            nc.vector.tensor_tensor(
                                out=ht[:, c * Q:c * Q + Q],
                                in0=xt[:, c * H:c * H + Q],
                                in1=xt[:, c * H + Q:(c + 1) * H],
                                op=OP.max,
                            )
                            tree16(ht, c * Q, Q)
                        nc.vector.tensor_tensor(
                            out=mh[:, k * T:(k + 1) * T], in0=ht[:, 0:T],
                            in1=ht[:, Q:Q + T], op=OP.max,
                        )
                if k == 1:
                    # lg (ACT Ln) is long done; wgt = -log(dup/8)/dup
                    nc.vector.scalar_tensor_tensor(
                        out=wgt[:, :], in0=lg[:, :], scalar=-1.0,
                        in1=rd[:, :], op0=OP.mult, op1=OP.mult,
                    )
                count(k)

            # ---- batched epilogue over all 8 tile columns ----
            nd = cp.tile([128, NT * L], f32)
            nsum = cp.tile([128, NT], f32)
            inv = cp.tile([128, NT], f32)
            npj = cp.tile([128, NT * L], f32)
            lj = cp.tile([128, NT * L], f32)
            nc.vector.tensor_mul(out=nd[:, :], in0=cnta[:, :], in1=rd[:, :])
            nc.vector.reduce_sum(
                out=nsum[:, :],
                in_=nd[:, :].rearrange("p (k j) -> p k j", j=L),
                axis=AX.X,
            )
            nc.vector.tensor_scalar_max(out=nsum[:, :], in0=nsum[:, :], scalar1=1.0)
            nc.vector.reciprocal(out=inv[:, :], in_=nsum[:, :])
            nc.vector.tensor_tensor(
                out=npj[:, :].rearrange("p (k j) -> p k j", j=L),
                in0=cnta[:, :].rearrange("p (k j) -> p k j", j=L),
                in1=inv[:, :].unsqueeze(2).to_broadcast([128, NT, L]),
                op=OP.mult,
            )
            nc.vector.tensor_scalar_max(out=npj[:, :], in0=npj[:, :], scalar1=EPS)
            nc.vector.tensor_mul(out=lj[:, :], in0=npj[:, :], in1=wgt[:, :])
            acc = cp.tile([128, 1], f32)
            nc.vector.reduce_sum(
                out=acc[:, :],
                in_=lj[:, :].rearrange("p (k j) -> p k j", j=L),
                axis=AX.XY,
            )
            # collapse partitions: PE dot with ones -> PSUM [1,1] -> SBUF
            psc = pp.tile([1, 1], f32)
            nc.tensor.matmul(psc[:, :], acc[:, :], ones[:, :],
                             start=True, stop=True)
            outv = cp.tile([1, 1], f32)
            nc.vector.tensor_copy(out=outv[:, :], in_=psc[:, :])
            nc.scalar.dma_start(out=out[:, :], in_=outv[:, :])
    nc.compile()
    return nc


def _shard_inputs(x, y, target_lengths):
    """Numpy-side sharding, target-row pre-gather, and device layouts."""
    x = np.ascontiguousarray(np.asarray(x, dtype=np.float32))
    y = np.asarray(y, dtype=np.int32)
    y2 = y.reshape(B, L)  # target_lengths is L for every sample (spec'd)
    x3 = x.reshape(B, C, T)
    xg_all = np.take_along_axis(
        x3, y2[:, :, None].astype(np.int64), axis=1
    ).astype(np.float16)

    in_maps = []
    for i in range(N_CORES):
        sl = slice(i * B_SH, (i + 1) * B_SH)
        xs = x[sl].reshape(B_SH, C * T)
        xgs = np.ascontiguousarray(
            xg_all[sl].reshape(NT, 128, L * T).transpose(1, 0, 2).reshape(128, -1)
        )
        ycs = np.ascontiguousarray(
            y2[sl].reshape(NT, 128, L).transpose(1, 0, 2).reshape(128, -1)
        )
        in_maps.append({"x": xs, "xg": xgs, "yc": ycs})
    return in_maps


def kernel(x, y, target_lengths):
    import sys
    if "/opt/trn_rl_repo" not in sys.path:
        sys.path.insert(0, "/opt/trn_rl_repo")
    from concourse.bass_utils import run_bass_kernel_spmd

    if "nc" not in _CACHE:
        _CACHE["nc"] = _build_nc()
    nc = _CACHE["nc"]

    in_maps = _shard_inputs(x, y, target_lengths)
    res = run_bass_kernel_spmd(nc, in_maps, core_ids=list(range(N_CORES)))
    total = np.float64(0.0)
    for r in res.results:
        total += np.float64(np.asarray(r["out"]).reshape(()))
    return np.float32(total / B)


# revision 9
# speedup vs baseline: 1.1649x; 1.1649x over previous
"""ACE loss kernel for TRN2, data-parallel over 8 NeuronCores.

Math (per sample b, with targets y[b, 0:8] and logits x[b, c, t]):
  m[b,t]   = max_c x[b,c,t]
  cnt[b,j] = #{t : x[b, y[b,j], t] == m[b,t]}        == n_k[b, y[b,j]] (no ties)
  dup[b,j] = multiplicity of y[b,j] within y[b,:]    == y_k[b, y[b,j]]
Only target classes contribute to the masked loss, so the full 128-bin
argmax histogram is never materialized:
  n_sum[b] = sum_j cnt/dup   (each distinct class counted once)
  n_p[b,j] = max(cnt / max(n_sum,1), EPS)
  loss[b]  = sum_j n_p * (-log(dup/8)) / dup
  out      = mean_b loss

v3: x streams through HWDGE (sync-engine ring) in raw f32 at the
measured ~434 GB/s fabric/HBM-read ceiling (SWDGE descriptor generation
starves under DVE shared-port locks -- the v1 pathology; HWDGE is RTL
and immune).  yc/xg ride the SAME ring, sequenced between tile-0's
quarters and tile 1: v2 put them on the concurrent scalar ring, which
round-robins per packet with Q1 and collapsed both rings to ~160 GB/s
for 8 us.  Tiles 1-4 are cast f32->fp16 by the Scalar/ACT engine
(dedicated SBUF ports, otherwise idle) and max-reduced by DVE in fp16
2x mode; tiles 0,5,6,7 do the first tree level as a DVE f32 TT-max
with fp16 output (monotone cast -> same numerics), which has lower
latency than cast-then-tree and so suits the stream's head and tail.
The last tile loads as 2 quarters + 4 eighths with a running fold so
only ~1.6 us of tree work follows the final bytes.  One batched
epilogue covers all 8 tile columns, the [128,1] loss vector collapses
via a PE dot with ones, and the scalar leaves through a 4-byte
single-descriptor DMA.  Each core returns one f32; the host sums 8 and
divides by B.
"""

import numpy as np

B, C, T, L = 8192, 128, 64, 8
N_CORES = 8
B_SH = B // N_CORES          # 1024 samples per core
NT = B_SH // 128             # 8 tiles of 128 samples
EPS = 1e-5

_CACHE = {}


def _build_nc():
    import os
    import sys
    if "/opt/trn_rl_repo" not in sys.path:
        sys.path.insert(0, "/opt/trn_rl_repo")
    from concourse import bacc, mybir
    from concourse.tile import TileContext

    f32 = mybir.dt.float32
    f16 = mybir.dt.float16
    AX = mybir.AxisListType
    OP = mybir.AluOpType

    CT = C * T            # 8192 elems per sample
    Q = CT // 4           # 2048
    E = CT // 8           # 1024
    H = CT // 2           # 4096

    nc = bacc.Bacc("TRN2")
    x = nc.declare_dram_parameter("x", [B_SH, CT], f32, isOutput=False)
    xg = nc.declare_dram_parameter("xg", [128, NT * L * T], f16, isOutput=False)
    yc = nc.declare_dram_parameter("yc", [128, NT * L], mybir.dt.int32, isOutput=False)
    out = nc.declare_dram_parameter("out", [1, 1], f32, isOutput=True)

    with TileContext(nc) as tc:
        with (
            tc.tile_pool(name="xp", bufs=4) as xp,
            tc.tile_pool(name="hp", bufs=2) as hp,
            tc.tile_pool(name="sp", bufs=2) as sp,
            tc.tile_pool(name="cp", bufs=1) as cp,
            tc.tile_pool(name="ps", bufs=1, space="PSUM") as pp,
        ):
            # ---- whole-run tiles ----
            xga = cp.tile([128, NT * L * T], f16)
            ycta = cp.tile([128, NT * L], mybir.dt.int32)
            mh = cp.tile([128, NT * T], f16)      # per-tile class-max rows
            cnta = cp.tile([128, NT * L], f32)
            ones = cp.tile([128, 1], f32)
            nc.gpsimd.memset(ones[:, :], 1.0)

            ycf = cp.tile([128, NT * L], f32)
            eq8 = cp.tile([128, NT * L * L], f32)
            dup = cp.tile([128, NT * L], f32)
            rd = cp.tile([128, NT * L], f32)
            lg = cp.tile([128, NT * L], f32)
            wgt = cp.tile([128, NT * L], f32)

            def ymath():
                nc.vector.tensor_copy(out=ycf[:, :], in_=ycta[:, :])
                nc.vector.tensor_tensor(
                    out=eq8[:, :].rearrange("p (k a b) -> p k a b", a=L, b=L),
                    in0=ycf[:, :].rearrange("p (k a) -> p k a", a=L)
                    .unsqueeze(3).to_broadcast([128, NT, L, L]),
                    in1=ycf[:, :].rearrange("p (k a) -> p k a", a=L)
                    .unsqueeze(2).to_broadcast([128, NT, L, L]),
                    op=OP.is_equal,
                )
                nc.vector.reduce_sum(
                    out=dup[:, :],
                    in_=eq8[:, :].rearrange("p (k a b) -> p k a b", a=L, b=L),
                    axis=AX.X,
                )
                nc.vector.reciprocal(out=rd[:, :], in_=dup[:, :])
                nc.scalar.activation(
                    out=lg[:, :], in_=dup[:, :],
                    func=mybir.ActivationFunctionType.Ln, scale=1.0 / L,
                )

            # f32 pair-max with fp16 out: xt[lo:lo+n] -> ht[hoff:hoff+n/2]
            def l1(xt, lo, n, ht, hoff):
                nc.vector.tensor_tensor(
                    out=ht[:, hoff:hoff + n // 2],
                    in0=xt[:, lo:lo + n // 2],
                    in1=xt[:, lo + n // 2:lo + n],
                    op=OP.max,
                )

            # fp16 in-place max tree over t[:, lo:lo+n] down to W wide;
            # if last_out is given (W == T), the final level writes there
            def tree16(t, lo, n, W=T, last_out=None):
                w = n
                while w > W:
                    h = w // 2
                    dst = (last_out if (last_out is not None and h == W)
                           else t[:, lo:lo + h])
                    nc.vector.tensor_tensor(
                        out=dst, in0=t[:, lo:lo + h],
                        in1=t[:, lo + h:lo + w], op=OP.max,
                    )
                    w = h

            def count(k):
                eq = sp.tile([128, L * T], f16, tag="eq")
                nc.vector.tensor_tensor(
                    out=eq[:, :].rearrange("p (l t) -> p l t", l=L),
                    in0=xga[:, k * L * T:(k + 1) * L * T].rearrange(
                        "p (l t) -> p l t", l=L
                    ),
                    in1=mh[:, k * T:(k + 1) * T].unsqueeze(1)
                    .to_broadcast([128, L, T]),
                    op=OP.is_equal,
                )
                nc.vector.reduce_sum(
                    out=cnta[:, k * L:(k + 1) * L],
                    in_=eq[:, :].rearrange("p (l t) -> p l t", l=L),
                    axis=AX.X,
                )

            for k in range(NT):
                row = slice(k * 128, (k + 1) * 128)
                xt = xp.tile([128, CT], f32, tag="xt")
                ht = hp.tile([128, H], f16, tag="ht")
                mcol = mh[:, k * T:(k + 1) * T]
                if k == 0:
                    # quarters with per-piece sub-trees: compute starts after
                    # the first MiB (short pipeline ramp)
                    for c in range(4):
                        nc.sync.dma_start(
                            out=xt[:, c * Q:(c + 1) * Q],
                            in_=x[row, c * Q:(c + 1) * Q],
                        )
                        if c == 3:
                            # yc + xg ride the x ring here, serialized after
                            # the quarters: on the concurrent scalar ring
                            # they halve Q1 throughput for ~8 us
                            nc.sync.dma_start(out=ycta[:, :], in_=yc[:, :])
                            nc.sync.dma_start(out=xga[:, :], in_=xg[:, :])
                        hoff = c * (Q // 2)
                        l1(xt, c * Q, Q, ht, hoff)
                        tree16(ht, hoff, Q // 2)
                        if c > 0:
                            dst = mcol if c == 3 else ht[:, 0:T]
                            nc.vector.tensor_tensor(
                                out=dst, in0=ht[:, 0:T],
                                in1=ht[:, hoff:hoff + T], op=OP.max,
                            )
                        if c == 3:
                            # yc landed right after the quarters; y-side math
                            # fills the DVE gap before xga arrives
                            ymath()
                elif k < NT - 2:
                    # mid tiles: whole-tile load, full DVE tree (fewest ops)
                    nc.sync.dma_start(out=xt[:, :], in_=x[row, :])
                    l1(xt, 0, CT, ht, 0)
                    tree16(ht, 0, H, T, last_out=mcol)
                elif k == NT - 2:
                    # halves with sub-trees folded at 512 wide (earlier
                    # start, cheap tail transition)
                    for c in range(2):
                        sl = slice(c * H, (c + 1) * H)
                        nc.sync.dma_start(out=xt[:, sl], in_=x[row, sl])
                        l1(xt, c * H, H, ht, c * Q)
                        tree16(ht, c * Q, Q, 512)
                    nc.vector.tensor_tensor(
                        out=ht[:, 0:512], in0=ht[:, 0:512],
                        in1=ht[:, Q:Q + 512], op=OP.max,
                    )
                    tree16(ht, 0, 512, T, last_out=mcol)
                else:
                    # last tile: 2 quarters + 4 eighths, running fold at 512
                    # wide -> only ~2.5 us of tree work follows the last byte
                    pieces = [(0, Q), (Q, Q)] + [(H + c * E, E) for c in range(4)]
                    hoff = 0
                    for pi, (lo, n) in enumerate(pieces):
                        nc.sync.dma_start(
                            out=xt[:, lo:lo + n], in_=x[row, lo:lo + n]
                        )
                        l1(xt, lo, n, ht, hoff)
                        tree16(ht, hoff, n // 2, 512)
                        if pi > 0:
                            nc.vector.tensor_tensor(
                                out=ht[:, 0:512], in0=ht[:, 0:512],
                                in1=ht[:, hoff:hoff + 512], op=OP.max,
                            )
                        hoff += n // 2
                    tree16(ht, 0, 512, T, last_out=mcol)
                if k == 1:
                    # lg (ACT Ln) is long done; wgt = -log(dup/8)/dup
                    nc.vector.scalar_tensor_tensor(
                        out=wgt[:, :], in0=lg[:, :], scalar=-1.0,
                        in1=rd[:, :], op0=OP.mult, op1=OP.mult,
                    )
                count(k)

            # ---- batched epilogue over all 8 tile columns ----
            nd = cp.tile([128, NT * L], f32)
            nsum = cp.tile([128, NT], f32)
            inv = cp.tile([128, NT], f32)
            npj = cp.tile([128, NT * L], f32)
            lj = cp.tile([128, NT * L], f32)
            nc.vector.tensor_mul(out=nd[:, :], in0=cnta[:, :], in1=rd[:, :])
            nc.vector.reduce_sum(
                out=nsum[:, :],
                in_=nd[:, :].rearrange("p (k j) -> p k j", j=L),
                axis=AX.X,
            )
            nc.vector.tensor_scalar_max(out=nsum[:, :], in0=nsum[:, :], scalar1=1.0)
            nc.vector.reciprocal(out=inv[:, :], in_=nsum[:, :])
            nc.vector.tensor_tensor(
                out=npj[:, :].rearrange("p (k j) -> p k j", j=L),
                in0=cnta[:, :].rearrange("p (k j) -> p k j", j=L),
                in1=inv[:, :].unsqueeze(2).to_broadcast([128, NT, L]),
                op=OP.mult,
            )
            nc.vector.tensor_scalar_max(out=npj[:, :], in0=npj[:, :], scalar1=EPS)
            nc.vector.tensor_mul(out=lj[:, :], in0=npj[:, :], in1=wgt[:, :])
            acc = cp.tile([128, 1], f32)
            nc.vector.reduce_sum(
                out=acc[:, :],
                in_=lj[:, :].rearrange("p (k j) -> p k j", j=L),
                axis=AX.XY,
            )
            # collapse partitions: PE dot with ones -> PSUM [1,1] -> SBUF
            psc = pp.tile([1, 1], f32)
            nc.tensor.matmul(psc[:, :], acc[:, :], ones[:, :],
                             start=True, stop=True)
            outv = cp.tile([1, 1], f32)
            nc.vector.tensor_copy(out=outv[:, :], in_=psc[:, :])
            nc.sync.dma_start(out=out[:, :], in_=outv[:, :])
    nc.compile()
    return nc


def _shard_inputs(x, y, target_lengths):
    """Numpy-side sharding, target-row pre-gather, and device layouts."""
    x = np.ascontiguousarray(np.asarray(x, dtype=np.float32))
    y = np.asarray(y, dtype=np.int32)
    y2 = y.reshape(B, L)  # target_lengths is L for every sample (spec'd)
    x3 = x.reshape(B, C, T)
    xg_all = np.take_along_axis(
        x3, y2[:, :, None].astype(np.int64), axis=1
    ).astype(np.float16)

    in_maps = []
    for i in range(N_CORES):
        sl = slice(i * B_SH, (i + 1) * B_SH)
        xs = x[sl].reshape(B_SH, C * T)
        xgs = np.ascontiguousarray(
            xg_all[sl].reshape(NT, 128, L * T).transpose(1, 0, 2).reshape(128, -1)
        )
        ycs = np.ascontiguousarray(
            y2[sl].reshape(NT, 128, L).transpose(1, 0, 2).reshape(128, -1)
        )
        in_maps.append({"x": xs, "xg": xgs, "yc": ycs})
    return in_maps


def kernel(x, y, target_lengths):
    import sys
    if "/opt/trn_rl_repo" not in sys.path:
        sys.path.insert(0, "/opt/trn_rl_repo")
    from concourse.bass_utils import run_bass_kernel_spmd

    if "nc" not in _CACHE:
        _CACHE["nc"] = _build_nc()
    nc = _CACHE["nc"]

    in_maps = _shard_inputs(x, y, target_lengths)
    res = run_bass_kernel_spmd(nc, in_maps, core_ids=list(range(N_CORES)))
    total = np.float64(0.0)
    for r in res.results:
        total += np.float64(np.asarray(r["out"]).reshape(()))
    return np.float32(total / B)
